# revision 13
# baseline (speedup 1.0000x reference)
"""Trainium2 Bass kernel for nn_CrossAttnPromptModel (8-core SPMD).

Sharding: core c -> (batch b=c//2, row-half c%2). Each core processes 688 of the
1369 image tokens for its batch (halves overlap by 7 rows); prompt-side work
(d2f MLP, ctx LN, K/V projection) is duplicated across the pair. The second
cross-attention and the ctx MLP do not affect the returned output (dead code)
and are skipped. Matmuls run in fp32r (TF32-like, full PE rate); the
attention AV and MLP second matmul run in bf16 to fit SBUF.
"""
import sys
if '/opt/trn_rl_repo' not in sys.path:
    sys.path.insert(0, '/opt/trn_rl_repo')
import numpy as np
import ml_dtypes
import concourse.bass as bass
import concourse.mybir as mybir
import concourse.tile as tile
from concourse import bacc
from concourse.bass_utils import run_bass_kernel_spmd
from concourse.masks import make_identity

f32 = mybir.dt.float32
f32r = mybir.dt.float32r
bf16 = mybir.dt.bfloat16
AF = mybir.ActivationFunctionType
ALU = mybir.AluOpType

PI = float(np.pi)
MAGIC = 1.5 * 2.0 ** 23
B, NFULL, DIM, HEADS, HEAD, PE = 4, 1369, 1024, 4, 256, 512
M, MT = 2304, 18
NQ, QT = 688, 344
RC = [(0, 128), (128, 128), (256, 128), (384, 128), (512, 128), (640, 48)]
TOKT = [(0, 512), (512, 512), (1024, 512), (1536, 512), (2048, 256)]
LN_EPS = 1e-5

_cached = None


def _build():
    nc = bacc.Bacc("TRN2", target_bir_lowering=False, debug=False)

    def din(name, shape, dt):
        return nc.dram_tensor(name, shape, dt, kind="ExternalInput").ap()

    ximg = din("ximg", [NQ, DIM], f32)
    cimg = din("cimg", [128, 6, 2], f32)
    cpmt = din("cpmt", [128, MT, 2], f32)
    gauss = din("gauss", [2, PE], f32)
    depth = din("depth", [1, M], f32r)
    maskv = din("maskv", [M], f32)
    wq = din("wq", [DIM, DIM], f32r)
    wk = din("wk", [DIM, DIM], f32r)
    wv = din("wv", [DIM, DIM], f32r)
    wo = din("wo", [DIM, DIM], f32r)
    w1 = din("w1", [DIM, 4 * DIM], f32r)
    w2b = din("w2b", [4 * DIM, DIM], bf16)
    d2fw2 = din("d2fw2", [PE, DIM], bf16)
    w1pp = din("w1pp", [128, 4], f32)
    b1pp = din("b1pp", [128, 4], f32)
    bqpp = din("bqpp", [128, 8], f32)
    bkpp = din("bkpp", [128, 8], f32)
    b1mpp = din("b1mpp", [128, 32], f32)
    VNAMES = ["n1cg", "n1cb", "n1xg", "n1xb", "n2xg", "n2xb", "bo_v", "bv_v", "b2f_v", "b2m_v"]
    vecs = {n: din(n, [1, DIM], f32) for n in VNAMES}
    out = nc.dram_tensor("out", [NQ, DIM], f32, kind="ExternalOutput").ap()
    dbg_cn = nc.dram_tensor("dbg_cn", [128, 8, 128], f32r, kind="ExternalOutput").ap()
    dbg_x = nc.dram_tensor("dbg_x", [128, DIM], f32, kind="ExternalOutput").ap()
    dbg_xn = nc.dram_tensor("dbg_xn", [128, 8, 128], f32r, kind="ExternalOutput").ap()
    dbg_q = nc.dram_tensor("dbg_q", [128, 8, 128], f32r, kind="ExternalOutput").ap()
    dbg_o = nc.dram_tensor("dbg_o", [128, 8, 128], f32r, kind="ExternalOutput").ap()
    dbg_k = nc.dram_tensor("dbg_k", [128, 1024], f32r, kind="ExternalOutput").ap()
    dbg_v = nc.dram_tensor("dbg_v", [128, DIM], bf16, kind="ExternalOutput").ap()
    kscr = nc.dram_tensor("kscr", [8, 128, M], f32r).ap()
    vscr = nc.dram_tensor("vscr", [M, DIM], bf16).ap()

    with tile.TileContext(nc) as tc:
        cp = tc.alloc_tile_pool(name="consts", bufs=1)
        ident_f = cp.tile([128, 128], f32)
        make_identity(nc, ident_f)
        ident = cp.tile([128, 128], f32r)
        nc.vector.tensor_copy(ident, ident_f)
        ones_f = cp.tile([128, 1], f32)
        nc.vector.memset(ones_f, 1.0)
        ones128b = cp.tile([128, 1], bf16)
        nc.vector.tensor_copy(ones128b, ones_f)
        onesrow_f = cp.tile([1, 128], f32)
        nc.vector.memset(onesrow_f, 1.0)
        onesrow = cp.tile([1, 128], f32r)
        nc.vector.tensor_copy(onesrow, onesrow_f)
        epst = cp.tile([128, 1], f32)
        nc.vector.memset(epst, LN_EPS)
        halfpi = cp.tile([128, 1], f32)
        nc.vector.memset(halfpi, PI / 2)
        g0B = cp.tile([128, PE], f32)
        nc.sync.dma_start(out=g0B, in_=gauss[0:1, :].broadcast_to([128, PE]))
        g1B = cp.tile([128, PE], f32)
        nc.sync.dma_start(out=g1B, in_=gauss[1:2, :].broadcast_to([128, PE]))
        cimg_sb = cp.tile([128, 6, 2], f32)
        nc.sync.dma_start(out=cimg_sb, in_=cimg)
        w1pp_sb = cp.tile([128, 4], f32)
        nc.sync.dma_start(out=w1pp_sb, in_=w1pp)
        b1pp_sb = cp.tile([128, 4], f32)
        nc.sync.dma_start(out=b1pp_sb, in_=b1pp)
        bqpp_sb = cp.tile([128, 8], f32)
        nc.sync.dma_start(out=bqpp_sb, in_=bqpp)
        bkpp_sb = cp.tile([128, 8], f32)
        nc.sync.dma_start(out=bkpp_sb, in_=bkpp)
        b1mpp_sb = cp.tile([128, 32], f32)
        nc.sync.dma_start(out=b1mpp_sb, in_=b1mpp)
        maskT = cp.tile([128, MT], f32)
        nc.sync.dma_start(out=maskT, in_=maskv.rearrange("(mt p) -> p mt", p=128))
        mbias = cp.tile([128, MT], f32)
        nc.vector.tensor_scalar(mbias, maskT, 1e-6, None, op0=ALU.min)
        nc.vector.tensor_scalar(mbias, mbias, 1e-6, 1e15, op0=ALU.subtract, op1=ALU.mult)

        def loadvec(pool, name):
            t = pool.tile([128, DIM], f32, tag=name)
            nc.sync.dma_start(out=t, in_=vecs[name].broadcast_to([128, DIM]))
            return t

        def pe_arg(pool, cpp, p, tag):
            arg = pool.tile([128, PE], f32, tag=tag + "a")
            t2 = pool.tile([128, PE], f32, tag=tag + "b")
            nc.gpsimd.tensor_scalar(arg[:p], g0B[:p], cpp[:p, 0:1], None, op0=ALU.mult)
            nc.gpsimd.tensor_scalar(t2[:p], g1B[:p], cpp[:p, 1:2], None, op0=ALU.mult)
            nc.gpsimd.tensor_tensor(out=arg[:p], in0=arg[:p], in1=t2[:p], op=ALU.add)
            return arg

        def sincos(pool, ps_arg, p, dst):
            # dst[:p, 0:512] = sin(arg); dst[:p, 512:1024] = cos(arg)
            t1 = pool.tile([128, PE], f32, tag="pet1")
            t2 = pool.tile([128, PE], f32, tag="pet2")
            z = pool.tile([128, PE], f32, tag="pez")
            nc.vector.tensor_scalar(t1[:p], ps_arg, 1.0 / (2 * PI), MAGIC, op0=ALU.mult, op1=ALU.add)
            nc.vector.tensor_scalar(t2[:p], t1[:p], MAGIC, 2 * PI, op0=ALU.subtract, op1=ALU.mult)
            nc.vector.tensor_tensor(out=z[:p], in0=ps_arg, in1=t2[:p], op=ALU.subtract)
            nc.scalar.activation(dst[:p, 0:PE], z[:p], AF.Sin)
            # cos(x) = sin(zc + pi/2), zc = x - 2*pi*round(x/(2*pi) + 1/4)
            nc.vector.tensor_scalar(t1[:p], ps_arg, 1.0 / (2 * PI), 0.25, op0=ALU.mult, op1=ALU.add)
            nc.vector.tensor_scalar(t1[:p], t1[:p], MAGIC, None, op0=ALU.add)
            nc.vector.tensor_scalar(t2[:p], t1[:p], MAGIC, 2 * PI, op0=ALU.subtract, op1=ALU.mult)
            nc.vector.tensor_tensor(out=z[:p], in0=ps_arg, in1=t2[:p], op=ALU.subtract)
            nc.scalar.activation(dst[:p, PE:2 * PE], z[:p], AF.Sin, bias=halfpi[:p])

        def ln_apply(pool, src, p, gB, bB, dst):
            # row-major layernorm of src [p, DIM] f32 -> dst [p, DIM] f32r
            stats = pool.tile([128, 2, nc.vector.BN_STATS_DIM], f32, tag="lnstats")
            nc.vector.bn_stats(out=stats[:p, 0, :], in_=src[:, 0:512])
            nc.vector.bn_stats(out=stats[:p, 1, :], in_=src[:, 512:1024])
            mv = pool.tile([128, nc.vector.BN_AGGR_DIM], f32, tag="lnmv")
            nc.vector.bn_aggr(out=mv[:p], in_=stats[:p])
            sd = pool.tile([128, 1], f32, tag="lnsd")
            nc.scalar.activation(sd[:p], mv[:p, 1:2], AF.Sqrt, bias=epst[:p])
            rstd = pool.tile([128, 1], f32, tag="lnrstd")
            nc.vector.reciprocal(rstd[:p], sd[:p])
            t = pool.tile([128, DIM], f32, tag="lnt")
            nc.vector.tensor_scalar(t[:p], src, mv[:p, 0:1], rstd[:p], op0=ALU.subtract, op1=ALU.mult)
            nc.gpsimd.tensor_tensor(out=t[:p], in0=t[:p], in1=gB[:p], op=ALU.mult)
            nc.gpsimd.tensor_tensor(out=dst[:p], in0=t[:p], in1=bB[:p], op=ALU.add)

        xp = tc.alloc_tile_pool(name="xpool", bufs=1)
        x_sb = xp.tile([128, 6, DIM], f32)

        # ---------------- phase P: prompt path -> cnT ----------------
        with tc.tile_pool(name="cnpool", bufs=1) as cnp:
            cnT = cnp.tile([128, 8, M], f32r)
            with tc.tile_pool(name="ppool", bufs=1) as pp, \
                 tc.tile_pool(name="pwork", bufs=2) as pw, \
                 tc.tile_pool(name="plnp", bufs=2) as plnp:
                cpmt_sb = pp.tile([128, MT, 2], f32)
                nc.sync.dma_start(out=cpmt_sb, in_=cpmt)
                n1cg = loadvec(pp, "n1cg")
                n1cb = loadvec(pp, "n1cb")
                b2fB = loadvec(pp, "b2f_v")
                h1T = pp.tile([128, 4, M], bf16)
                with tc.tile_pool(name="pdep", bufs=1) as pdp, \
                     tc.tile_pool(name="psd", bufs=2, space="PSUM") as psdp:
                    depth_sb = pdp.tile([1, M], f32r)
                    nc.sync.dma_start(out=depth_sb, in_=depth)
                    for (o, w) in TOKT:
                        psd = psdp.tile([128, 512], f32, tag="psd")
                        nc.tensor.matmul(psd[:, :w], onesrow, depth_sb[:, o:o + w], start=True, stop=True)
                        for fc in range(4):
                            nc.scalar.activation(h1T[:, fc, o:o + w], psd[:, :w], AF.Gelu,
                                                 scale=w1pp_sb[:, fc:fc + 1], bias=b1pp_sb[:, fc:fc + 1])
                d2f_c = []
                for fc in range(4):
                    t = pp.tile([128, DIM], bf16, tag=f"d2f{fc}")
                    nc.sync.dma_start(out=t, in_=d2fw2[fc * 128:(fc + 1) * 128, :])
                    d2f_c.append(t)
                with tc.tile_pool(name="pemb", bufs=3, space="PSUM") as pembp, \
                     tc.tile_pool(name="ptr", bufs=3, space="PSUM") as ptrp:
                    for mt in range(MT):
                        # positional-encoding args for this token chunk
                        ps_arg = pe_arg(pw, cpmt_sb[:, mt, :], 128, "parg")
                        ctx_c = pw.tile([128, DIM], f32, tag="ctxc")
                        sincos(pw, ps_arg, 128, ctx_c)
                        # prompt_emb (d2f) for this chunk, add into ctx
                        for dh in range(2):
                            ps_e = pembp.tile([128, 512], f32, tag="pemb")
                            for fc in range(4):
                                nc.tensor.matmul(ps_e, h1T[:, fc, mt * 128:(mt + 1) * 128],
                                                 d2f_c[fc][:, dh * 512:(dh + 1) * 512],
                                                 start=(fc == 0), stop=(fc == 3))
                            t_e = pw.tile([128, 512], f32, tag="tembs")
                            nc.vector.tensor_tensor(out=t_e, in0=ps_e, in1=b2fB[:, dh * 512:(dh + 1) * 512], op=ALU.add)
                            nc.vector.tensor_tensor(out=ctx_c[:, dh * 512:(dh + 1) * 512],
                                                    in0=ctx_c[:, dh * 512:(dh + 1) * 512], in1=t_e, op=ALU.add)
                        cn_c = pw.tile([128, DIM], f32r, tag="cnc")
                        ln_apply(plnp, ctx_c, 128, n1cg, n1cb, cn_c)
                        for fc in range(8):
                            ps_t = ptrp.tile([128, 128], f32r, tag="ptr")
                            nc.tensor.transpose(ps_t, cn_c[:, fc * 128:(fc + 1) * 128], ident)
                            nc.vector.tensor_copy(cnT[:, fc, mt * 128:(mt + 1) * 128], ps_t)

            # ---------------- phase K: K/V projection (spilled to DRAM) ----------
            with tc.tile_pool(name="kvv", bufs=1) as kvv, \
                 tc.tile_pool(name="kst", bufs=3) as kstp, \
                 tc.tile_pool(name="pk", bufs=3, space="PSUM") as pkp:
                bvB = loadvec(kvv, "bv_v")
                with tc.tile_pool(name="wkp", bufs=1) as wkp:
                    wk_c = []
                    for kc in range(8):
                        t = wkp.tile([128, DIM], f32r, tag=f"wk{kc}")
                        nc.sync.dma_start(out=t, in_=wk[kc * 128:(kc + 1) * 128, :])
                        wk_c.append(t)
                    for fc in range(8):
                        for (o, w) in TOKT:
                            ps_k = pkp.tile([128, 512], f32, tag="pk")
                            for kc in range(8):
                                nc.tensor.matmul(ps_k[:, :w], wk_c[kc][:, fc * 128:(fc + 1) * 128],
                                                 cnT[:, kc, o:o + w], start=(kc == 0), stop=(kc == 7))
                            kst = kstp.tile([128, 512], f32r, tag="kst")
                            nc.vector.tensor_scalar(kst[:, :w], ps_k[:, :w], bkpp_sb[:, fc:fc + 1], None, op0=ALU.add)
                            nc.sync.dma_start(out=kscr[fc, :, o:o + w], in_=kst[:, :w])
                with tc.tile_pool(name="wvp", bufs=1) as wvp:
                    wv_c = []
                    for kc in range(8):
                        t = wvp.tile([128, DIM], f32r, tag=f"wv{kc}")
                        nc.sync.dma_start(out=t, in_=wv[kc * 128:(kc + 1) * 128, :])
                        wv_c.append(t)
                    for mt in range(MT):
                        vst = kstp.tile([128, DIM], bf16, tag="vst")
                        for dh in range(2):
                            ps_v = pkp.tile([128, 512], f32, tag="pv")
                            for kc in range(8):
                                nc.tensor.matmul(ps_v, cnT[:, kc, mt * 128:(mt + 1) * 128],
                                                 wv_c[kc][:, dh * 512:(dh + 1) * 512],
                                                 start=(kc == 0), stop=(kc == 7))
                            nc.vector.tensor_tensor(out=vst[:, dh * 512:(dh + 1) * 512], in0=ps_v,
                                                    in1=bvB[:, dh * 512:(dh + 1) * 512], op=ALU.add)
                        nc.sync.dma_start(out=vscr[mt * 128:(mt + 1) * 128, :], in_=vst)
                    nc.sync.dma_start(out=dbg_cn, in_=cnT[:, :, 0:128])

        qo = tc.alloc_tile_pool(name="qopool", bufs=1)
        qT = qo.tile([128, 8, NQ], f32r)
        oT = qo.tile([128, 8, NQ], f32r)

        # ---------------- phase X: image path -> x_sb, qT --------------------
        with tc.tile_pool(name="xw", bufs=1) as xw, \
             tc.tile_pool(name="xwork", bufs=3) as xwk, \
             tc.tile_pool(name="xlnp", bufs=2) as xlnp:
            n1xg = loadvec(xw, "n1xg")
            n1xb = loadvec(xw, "n1xb")
            xnT = xw.tile([128, 8, NQ], f32r)
            with tc.tile_pool(name="xnrm", bufs=1) as xnrmp, \
                 tc.tile_pool(name="xtr", bufs=4, space="PSUM") as xtrp:
                xn_rm = xnrmp.tile([128, 6, DIM], f32r)
                for rc, (o, p) in enumerate(RC):
                    nc.sync.dma_start(out=x_sb[:p, rc, :], in_=ximg[o:o + p, :])
                    ps_arg = pe_arg(xwk, cimg_sb[:, rc, :], p, "xarg")
                    pe_c = xwk.tile([128, DIM], f32, tag="pec")
                    sincos(xwk, ps_arg[:p], p, pe_c)
                    nc.gpsimd.tensor_tensor(out=x_sb[:p, rc, :], in0=x_sb[:p, rc, :], in1=pe_c[:p], op=ALU.add)
                    ln_apply(xlnp, x_sb[:p, rc, :], p, n1xg, n1xb, xn_rm[:, rc, :])
                for rc, (o, p) in enumerate(RC):
                    for fc in range(8):
                        ps_t = xtrp.tile([128, 128], f32r, tag="xtr")
                        nc.tensor.transpose(ps_t[:, :p], xn_rm[:p, rc, fc * 128:(fc + 1) * 128], ident[:p, :p])
                        nc.vector.tensor_copy(xnT[:, fc, o:o + p], ps_t[:, :p])
                nc.sync.dma_start(out=dbg_x, in_=x_sb[:, 0, :])
                nc.sync.dma_start(out=dbg_xn, in_=xnT[:, :, 0:128])
            with tc.tile_pool(name="wqp", bufs=1) as wqp, \
                 tc.tile_pool(name="pq", bufs=3, space="PSUM") as pqp:
                wq_c = []
                for kc in range(8):
                    t = wqp.tile([128, DIM], f32r, tag=f"wq{kc}")
                    nc.sync.dma_start(out=t, in_=wq[kc * 128:(kc + 1) * 128, :])
                    wq_c.append(t)
                for fc in range(8):
                    for qt in range(2):
                        ps_q = pqp.tile([128, QT], f32, tag="pq")
                        for kc in range(8):
                            nc.tensor.matmul(ps_q, wq_c[kc][:, fc * 128:(fc + 1) * 128],
                                             xnT[:, kc, qt * QT:(qt + 1) * QT], start=(kc == 0), stop=(kc == 7))
                        nc.vector.tensor_scalar(qT[:, fc, qt * QT:(qt + 1) * QT], ps_q,
                                                bqpp_sb[:, fc:fc + 1], None, op0=ALU.add)
                nc.sync.dma_start(out=dbg_q, in_=qT[:, :, 0:128])
                nc.sync.dma_start(out=dbg_k, in_=kscr[0, :, 0:1024])
                nc.sync.dma_start(out=dbg_v, in_=vscr[0:128, :])

        # ---------------- phase A: attention ---------------------------------
        with tc.tile_pool(name="ah", bufs=2) as ahp, \
             tc.tile_pool(name="aex", bufs=1) as aexp, \
             tc.tile_pool(name="awk", bufs=3) as awk, \
             tc.tile_pool(name="pl", bufs=3, space="PSUM") as plp, \
             tc.tile_pool(name="psm", bufs=1, space="PSUM") as psmp, \
             tc.tile_pool(name="pav", bufs=2, space="PSUM") as pavp:
            for h in range(HEADS):
                kh = ahp.tile([128, 2, M], f32r, tag="kh")
                nc.sync.dma_start(out=kh, in_=kscr[2 * h:2 * h + 2].rearrange("c p m -> p c m"))
                vh = ahp.tile([128, MT, HEAD], bf16, tag="vh")
                nc.sync.dma_start(out=vh, in_=vscr.rearrange("(mt p) d -> p mt d", p=128)[:, :, h * HEAD:(h + 1) * HEAD])
                for qt in range(2):
                    expT = aexp.tile([128, MT, QT], bf16, tag="expT")
                    for mt in range(MT):
                        ps_l = plp.tile([128, QT], f32, tag="pl")
                        for c in range(2):
                            nc.tensor.matmul(ps_l, kh[:, c, mt * 128:(mt + 1) * 128],
                                             qT[:, 2 * h + c, qt * QT:(qt + 1) * QT],
                                             start=(c == 0), stop=(c == 1))
                        nc.scalar.activation(expT[:, mt, :], ps_l, AF.Exp,
                                             scale=0.0625, bias=mbias[:, mt:mt + 1])
                    ps_s = psmp.tile([1, QT], f32, tag="ps_s")
                    for mt in range(MT):
                        nc.tensor.matmul(ps_s, ones128b, expT[:, mt, :], start=(mt == 0), stop=(mt == MT - 1))
                    rr = awk.tile([1, QT], f32r, tag="rr")
                    with nc.allow_low_precision(reason="softmax denom reciprocal in f32r (~2^-12)"):
                        nc.vector.reciprocal(rr, ps_s)
                    ps_rb = psmp.tile([128, QT], f32, tag="ps_rb")
                    nc.tensor.matmul(ps_rb, onesrow, rr, start=True, stop=True)
                    recipB = awk.tile([128, QT], f32, tag="recipB")
                    nc.vector.tensor_copy(recipB, ps_rb)
                    for c in range(2):
                        ps_o = pavp.tile([128, QT], f32, tag="ps_o")
                        for mt in range(MT):
                            nc.tensor.matmul(ps_o, vh[:, mt, c * 128:(c + 1) * 128], expT[:, mt, :],
                                             start=(mt == 0), stop=(mt == MT - 1))
                        nc.vector.tensor_tensor(out=oT[:, 2 * h + c, qt * QT:(qt + 1) * QT],
                                                in0=ps_o, in1=recipB, op=ALU.mult)

            nc.sync.dma_start(out=dbg_o, in_=oT[:, :, 0:128])

        # attention out-projection + residual (row-major, into x_sb)
        with tc.tile_pool(name="wop", bufs=1) as wop, \
             tc.tile_pool(name="aprw", bufs=3) as aprw, \
             tc.tile_pool(name="pa", bufs=4, space="PSUM") as pap:
            boB = loadvec(wop, "bo_v")
            wo_c = []
            for fc in range(8):
                t = wop.tile([128, DIM], f32r, tag=f"wo{fc}")
                nc.sync.dma_start(out=t, in_=wo[fc * 128:(fc + 1) * 128, :])
                wo_c.append(t)
            for rc, (o, p) in enumerate(RC):
                for dh in range(2):
                    ps_a = pap.tile([128, 512], f32, tag="pa")
                    for fc in range(8):
                        nc.tensor.matmul(ps_a[:p], oT[:, fc, o:o + p],
                                         wo_c[fc][:, dh * 512:(dh + 1) * 512], start=(fc == 0), stop=(fc == 7))
                    t_a = aprw.tile([128, 512], f32, tag="ta")
                    nc.vector.tensor_tensor(out=t_a[:p], in0=ps_a[:p],
                                            in1=boB[:p, dh * 512:(dh + 1) * 512], op=ALU.add)
                    nc.vector.tensor_tensor(out=x_sb[:p, rc, dh * 512:(dh + 1) * 512],
                                            in0=x_sb[:p, rc, dh * 512:(dh + 1) * 512], in1=t_a[:p], op=ALU.add)

        qo.release()

        # ---------------- phase M: MLP on x2 ---------------------------------
        with tc.tile_pool(name="mw2", bufs=1) as mw2p, \
             tc.tile_pool(name="mmain", bufs=1) as mmain, \
             tc.tile_pool(name="mwork", bufs=2) as mwork:
            w2_c = []
            for hc in range(32):
                t = mw2p.tile([128, DIM], bf16, tag=f"w2{hc}")
                nc.sync.dma_start(out=t, in_=w2b[hc * 128:(hc + 1) * 128, :])
                w2_c.append(t)
            b2mB = loadvec(mmain, "b2m_v")
            x2nT = mmain.tile([128, 8, NQ], f32r)
            with tc.tile_pool(name="mlnv", bufs=1) as mlnv, \
                 tc.tile_pool(name="mlnp", bufs=2) as mlnp, \
                 tc.tile_pool(name="mtr", bufs=4, space="PSUM") as mtrp:
                n2xg = loadvec(mlnv, "n2xg")
                n2xb = loadvec(mlnv, "n2xb")
                x2n_rm = mlnv.tile([128, 6, DIM], f32r)
                for rc, (o, p) in enumerate(RC):
                    ln_apply(mlnp, x_sb[:p, rc, :], p, n2xg, n2xb, x2n_rm[:, rc, :])
                for rc, (o, p) in enumerate(RC):
                    for fc in range(8):
                        ps_t = mtrp.tile([128, 128], f32r, tag="mtr")
                        nc.tensor.transpose(ps_t[:, :p], x2n_rm[:p, rc, fc * 128:(fc + 1) * 128], ident[:p, :p])
                        nc.vector.tensor_copy(x2nT[:, fc, o:o + p], ps_t[:, :p])
            hTp = tc.alloc_tile_pool(name="hTp", bufs=1)
            hT = hTp.tile([128, 32, NQ], bf16)
            with tc.tile_pool(name="w1p", bufs=2) as w1p, \
                 tc.tile_pool(name="ph", bufs=4, space="PSUM") as php:
                for e in range(16):  # stream w1 in 1MB column blocks (2 h-chunks each)
                    w1_sb = w1p.tile([128, 8, 256], f32r, tag="w1s")
                    nc.sync.dma_start(out=w1_sb,
                                      in_=w1.rearrange("(c p) d -> p c d", p=128)[:, :, e * 256:(e + 1) * 256])
                    for hcl in range(2):
                        hc = e * 2 + hcl
                        for qt in range(2):
                            ps_h = php.tile([128, QT], f32, tag="ph")
                            for kc in range(8):
                                nc.tensor.matmul(ps_h, w1_sb[:, kc, hcl * 128:(hcl + 1) * 128],
                                                 x2nT[:, kc, qt * QT:(qt + 1) * QT],
                                                 start=(kc == 0), stop=(kc == 7))
                            nc.scalar.activation(hT[:, hc, qt * QT:(qt + 1) * QT], ps_h, AF.Gelu,
                                                 bias=b1mpp_sb[:, hc:hc + 1])
            with tc.tile_pool(name="po", bufs=4, space="PSUM") as pop:
                for rc, (o, p) in enumerate(RC):
                    for dh in range(2):
                        ps_f = pop.tile([128, 512], f32, tag="po")
                        for hc in range(32):
                            nc.tensor.matmul(ps_f[:p], hT[:, hc, o:o + p],
                                             w2_c[hc][:, dh * 512:(dh + 1) * 512],
                                             start=(hc == 0), stop=(hc == 31))
                        o_f = mwork.tile([128, 512], f32, tag="of")
                        nc.vector.tensor_tensor(out=o_f[:p], in0=ps_f[:p],
                                                in1=b2mB[:p, dh * 512:(dh + 1) * 512], op=ALU.add)
                        nc.vector.tensor_tensor(out=o_f[:p], in0=o_f[:p],
                                                in1=x_sb[:p, rc, dh * 512:(dh + 1) * 512], op=ALU.add)
                        nc.sync.dma_start(out=out[o:o + p, dh * 512:(dh + 1) * 512], in_=o_f[:p])

            hTp.release()
        xp.release()
        cp.release()

    nc.finalize()
    return nc


def _gridc(h, w):
    x = (np.arange(w, dtype=np.float64) + 0.5) / w
    y = (np.arange(h, dtype=np.float64) + 0.5) / h
    xx, yy = np.meshgrid(x, y)
    return np.stack([xx.ravel(), yy.ravel()], 0)  # [2, h*w], (x, y)


def _run(inputs, trace=False):
    global _cached
    if _cached is None:
        _cached = _build()
    nc = _cached

    p = {k: np.asarray(v, np.float32) for k, v in inputs["params"].items()}
    image = np.asarray(inputs["image_embeddings"], np.float32)
    pdepth = np.asarray(inputs["prompt_depth"], np.float32)
    pmask = np.asarray(inputs["prompt_mask"], np.float32)

    cimg_full = (2 * PI * (2 * _gridc(37, 37) - 1)).astype(np.float32)  # [2, 1369]
    cpmt_flat = (2 * PI * (2 * _gridc(48, 48) - 1)).astype(np.float32)  # [2, 2304]
    cpmt = np.ascontiguousarray(cpmt_flat.T.reshape(MT, 128, 2).transpose(1, 0, 2))  # [128, MT, 2]

    shared = {
        "cpmt": cpmt,
        "gauss": np.ascontiguousarray(p["pe_gauss"]),
        "wq": np.ascontiguousarray(p["ax_wq"]),
        "wk": np.ascontiguousarray(p["ax_wkv"][:, :DIM]),
        "wv": np.ascontiguousarray(p["ax_wkv"][:, DIM:]),
        "wo": np.ascontiguousarray(p["ax_wo"]),
        "w1": np.ascontiguousarray(p["mx_w1"]),
        "w2b": p["mx_w2"].astype(ml_dtypes.bfloat16),
        "d2fw2": p["d2f_w2"].astype(ml_dtypes.bfloat16),
        "w1pp": np.ascontiguousarray(p["d2f_w1"].reshape(4, 128).T),
        "b1pp": np.ascontiguousarray(p["d2f_b1"].reshape(4, 128).T),
        "bqpp": np.ascontiguousarray(p["ax_bq"].reshape(8, 128).T),
        "bkpp": np.ascontiguousarray(p["ax_bkv"][:DIM].reshape(8, 128).T),
        "b1mpp": np.ascontiguousarray(p["mx_b1"].reshape(32, 128).T),
        "n1cg": p["n1c_g"][None], "n1cb": p["n1c_b"][None],
        "n1xg": p["n1x_g"][None], "n1xb": p["n1x_b"][None],
        "n2xg": p["n2x_g"][None], "n2xb": p["n2x_b"][None],
        "bo_v": p["ax_bo"][None], "bv_v": p["ax_bkv"][None, DIM:],
        "b2f_v": p["d2f_b2"][None], "b2m_v": p["mx_b2"][None],
    }
    in_maps = []
    for c in range(8):
        b, half = c // 2, c % 2
        rows = slice(0, NQ) if half == 0 else slice(NFULL - NQ, NFULL)
        m = dict(shared)
        m["ximg"] = np.ascontiguousarray(image[b, rows])
        cr = np.zeros((768, 2), np.float32)
        cr[:NQ] = cimg_full[:, rows].T
        m["cimg"] = np.ascontiguousarray(cr.reshape(6, 128, 2).transpose(1, 0, 2))  # [128, 6, 2]
        m["depth"] = np.ascontiguousarray(pdepth[b, 0].reshape(1, M))
        m["maskv"] = np.ascontiguousarray(pmask[b, 0].reshape(M))
        in_maps.append(m)

    res = run_bass_kernel_spmd(nc, in_maps, list(range(8)), trace=trace,
                               stitch_traces=trace)
    full = np.empty((B, NFULL, DIM), np.float32)
    for b in range(B):
        full[b, :NFULL - NQ] = res.results[2 * b]["out"][:NFULL - NQ]
        full[b, NFULL - NQ:] = res.results[2 * b + 1]["out"]
    return full, res


def kernel(**inputs):
    full, _ = _run(inputs, trace=False)
    return full


# revision 20
# speedup vs baseline: 1.4626x; 1.4626x over previous
"""Trainium2 Bass kernel for nn_CrossAttnPromptModel (8-core SPMD).

Sharding: core c -> (batch b=c//2, row-half c%2). Each core processes 688 of the
1369 image tokens for its batch (halves overlap by 7 rows); prompt-side work
(d2f MLP, ctx LN, K/V projection) is duplicated across the pair. The second
cross-attention and the ctx MLP do not affect the returned output (dead code)
and are skipped. Matmuls run in fp32r (TF32-like, full PE rate); the
attention AV and MLP second matmul run in bf16 to fit SBUF.
"""
import sys
if '/opt/trn_rl_repo' not in sys.path:
    sys.path.insert(0, '/opt/trn_rl_repo')
import numpy as np
import ml_dtypes
import concourse.bass as bass
import concourse.mybir as mybir
import concourse.tile as tile
from concourse import bacc
from concourse.bass_utils import run_bass_kernel_spmd
from concourse.masks import make_identity

f32 = mybir.dt.float32
f32r = mybir.dt.float32r
bf16 = mybir.dt.bfloat16
AF = mybir.ActivationFunctionType
ALU = mybir.AluOpType

PI = float(np.pi)
MAGIC = 1.5 * 2.0 ** 23
B, NFULL, DIM, HEADS, HEAD, PE = 4, 1369, 1024, 4, 256, 512
M, MT = 2304, 18
NQ, QT = 688, 344
RC = [(0, 128), (128, 128), (256, 128), (384, 128), (512, 128), (640, 48)]
TOKT = [(0, 512), (512, 512), (1024, 512), (1536, 512), (2048, 256)]
LN_EPS = 1e-5

_cached = None


def _build():
    nc = bacc.Bacc("TRN2", target_bir_lowering=False, debug=False)

    def din(name, shape, dt):
        return nc.dram_tensor(name, shape, dt, kind="ExternalInput").ap()

    ximg = din("ximg", [NQ, DIM], f32)
    cimg = din("cimg", [128, 6, 2], f32)
    cpmt = din("cpmt", [128, MT, 2], f32)
    gauss = din("gauss", [2, PE], f32)
    depth = din("depth", [1, M], f32r)
    maskv = din("maskv", [M], f32)
    wq = din("wq", [DIM, DIM], f32r)
    wk = din("wk", [DIM, DIM], f32r)
    wv = din("wv", [DIM, DIM], f32r)
    wo = din("wo", [DIM, DIM], f32r)
    w1 = din("w1", [DIM, 4 * DIM], f32r)
    w2b = din("w2b", [4 * DIM, DIM], bf16)
    b2fr = din("b2fr", [1, DIM], f32r)
    d2fw2 = din("d2fw2", [PE, DIM], bf16)
    w1pp = din("w1pp", [128, 4], f32)
    b1pp = din("b1pp", [128, 4], f32)
    bqpp = din("bqpp", [128, 8], f32)
    bkpp = din("bkpp", [128, 8], f32)
    b1mpp = din("b1mpp", [128, 32], f32)
    VNAMES = ["n1cg", "n1cb", "n1xg", "n1xb", "n2xg", "n2xb", "bo_v", "bv_v", "b2f_v", "b2m_v"]
    vecs = {n: din(n, [1, DIM], f32) for n in VNAMES}
    out = nc.dram_tensor("out", [NQ, DIM], f32, kind="ExternalOutput").ap()
    dbg_cn = nc.dram_tensor("dbg_cn", [128, 8, 128], f32r, kind="ExternalOutput").ap()
    dbg_x = nc.dram_tensor("dbg_x", [128, DIM], f32, kind="ExternalOutput").ap()
    dbg_xn = nc.dram_tensor("dbg_xn", [128, 8, 128], f32r, kind="ExternalOutput").ap()
    dbg_q = nc.dram_tensor("dbg_q", [128, 8, 128], f32r, kind="ExternalOutput").ap()
    dbg_o = nc.dram_tensor("dbg_o", [128, 8, 128], f32r, kind="ExternalOutput").ap()
    dbg_k = nc.dram_tensor("dbg_k", [128, 1024], f32r, kind="ExternalOutput").ap()
    dbg_v = nc.dram_tensor("dbg_v", [128, DIM], bf16, kind="ExternalOutput").ap()
    kscr = nc.dram_tensor("kscr", [8, 128, M], f32r).ap()
    vscr = nc.dram_tensor("vscr", [M, DIM], bf16).ap()

    with tile.TileContext(nc) as tc:
        cp = tc.alloc_tile_pool(name="consts", bufs=1)
        ident_f = cp.tile([128, 128], f32)
        make_identity(nc, ident_f)
        ident = cp.tile([128, 128], f32r)
        nc.vector.tensor_copy(ident, ident_f)
        ones_f = cp.tile([128, 1], f32)
        nc.vector.memset(ones_f, 1.0)
        ones128b = cp.tile([128, 1], bf16)
        nc.vector.tensor_copy(ones128b, ones_f)
        onesrow_f = cp.tile([1, 128], f32)
        nc.vector.memset(onesrow_f, 1.0)
        onesrow = cp.tile([1, 128], f32r)
        nc.vector.tensor_copy(onesrow, onesrow_f)
        epst = cp.tile([128, 1], f32)
        nc.vector.memset(epst, LN_EPS)
        halfpi = cp.tile([128, 1], f32)
        nc.vector.memset(halfpi, PI / 2)
        g0B = cp.tile([128, PE], f32)
        nc.sync.dma_start(out=g0B, in_=gauss[0:1, :].broadcast_to([128, PE]))
        g1B = cp.tile([128, PE], f32)
        nc.sync.dma_start(out=g1B, in_=gauss[1:2, :].broadcast_to([128, PE]))
        cimg_sb = cp.tile([128, 6, 2], f32)
        nc.sync.dma_start(out=cimg_sb, in_=cimg)
        w1pp_sb = cp.tile([128, 4], f32)
        nc.sync.dma_start(out=w1pp_sb, in_=w1pp)
        b1pp_sb = cp.tile([128, 4], f32)
        nc.sync.dma_start(out=b1pp_sb, in_=b1pp)
        bqpp_sb = cp.tile([128, 8], f32)
        nc.sync.dma_start(out=bqpp_sb, in_=bqpp)
        bkpp_sb = cp.tile([128, 8], f32)
        nc.sync.dma_start(out=bkpp_sb, in_=bkpp)
        b1mpp_sb = cp.tile([128, 32], f32)
        nc.sync.dma_start(out=b1mpp_sb, in_=b1mpp)
        maskT = cp.tile([128, MT], f32)
        nc.sync.dma_start(out=maskT, in_=maskv.rearrange("(mt p) -> p mt", p=128))
        mbias = cp.tile([128, MT], f32)
        nc.vector.tensor_scalar(mbias, maskT, 1e-6, None, op0=ALU.min)
        nc.vector.tensor_scalar(mbias, mbias, 1e-6, 1e15, op0=ALU.subtract, op1=ALU.mult)

        def loadvec(pool, name):
            t = pool.tile([128, DIM], f32, tag=name)
            nc.sync.dma_start(out=t, in_=vecs[name].broadcast_to([128, DIM]))
            return t

        def pe_arg(pool, cpp, p, tag):
            arg = pool.tile([128, PE], f32, tag=tag + "a")
            t2 = pool.tile([128, PE], f32, tag=tag + "b")
            nc.vector.tensor_scalar(arg[:p], g0B[:p], cpp[:p, 0:1], None, op0=ALU.mult)
            nc.vector.tensor_scalar(t2[:p], g1B[:p], cpp[:p, 1:2], None, op0=ALU.mult)
            nc.vector.tensor_tensor(out=arg[:p], in0=arg[:p], in1=t2[:p], op=ALU.add)
            return arg

        def sincos(pool, ps_arg, p, dst):
            # dst[:p, 0:512] = sin(arg); dst[:p, 512:1024] = cos(arg)
            t1 = pool.tile([128, PE], f32, tag="pet1")
            t2 = pool.tile([128, PE], f32, tag="pet2")
            z = pool.tile([128, PE], f32, tag="pez")
            nc.vector.tensor_scalar(t1[:p], ps_arg, 1.0 / (2 * PI), MAGIC, op0=ALU.mult, op1=ALU.add)
            nc.vector.tensor_scalar(t2[:p], t1[:p], MAGIC, 2 * PI, op0=ALU.subtract, op1=ALU.mult)
            nc.vector.tensor_tensor(out=z[:p], in0=ps_arg, in1=t2[:p], op=ALU.subtract)
            nc.scalar.activation(dst[:p, 0:PE], z[:p], AF.Sin)
            # cos(x) = sin(zc + pi/2), zc = x - 2*pi*round(x/(2*pi) + 1/4)
            nc.vector.tensor_scalar(t1[:p], ps_arg, 1.0 / (2 * PI), 0.25, op0=ALU.mult, op1=ALU.add)
            nc.vector.tensor_scalar(t1[:p], t1[:p], MAGIC, None, op0=ALU.add)
            nc.vector.tensor_scalar(t2[:p], t1[:p], MAGIC, 2 * PI, op0=ALU.subtract, op1=ALU.mult)
            nc.vector.tensor_tensor(out=z[:p], in0=ps_arg, in1=t2[:p], op=ALU.subtract)
            nc.scalar.activation(dst[:p, PE:2 * PE], z[:p], AF.Sin, bias=halfpi[:p])

        def ln_apply(pool, src, p, dst):
            # row-major layernorm (affine folded into downstream weights):
            # dst = (src - mean) * rstd, f32 -> f32r
            stats = pool.tile([128, 2, nc.vector.BN_STATS_DIM], f32, tag="lnstats")
            nc.vector.bn_stats(out=stats[:p, 0, :], in_=src[:, 0:512])
            nc.vector.bn_stats(out=stats[:p, 1, :], in_=src[:, 512:1024])
            mv = pool.tile([128, nc.vector.BN_AGGR_DIM], f32, tag="lnmv")
            nc.vector.bn_aggr(out=mv[:p], in_=stats[:p])
            sd = pool.tile([128, 1], f32, tag="lnsd")
            nc.scalar.activation(sd[:p], mv[:p, 1:2], AF.Sqrt, bias=epst[:p])
            rstd = pool.tile([128, 1], f32, tag="lnrstd")
            nc.vector.reciprocal(rstd[:p], sd[:p])
            nc.vector.tensor_scalar(dst[:p], src, mv[:p, 0:1], rstd[:p], op0=ALU.subtract, op1=ALU.mult)

        xp = tc.alloc_tile_pool(name="xpool", bufs=1)
        x_sb = xp.tile([128, 6, DIM], f32)

        # ---------------- phase P: prompt path -> cnT ----------------
        with tc.tile_pool(name="cnpool", bufs=1) as cnp:
            cnT = cnp.tile([128, 8, M], f32r)
            with tc.tile_pool(name="ppool", bufs=1) as pp, \
                 tc.tile_pool(name="pwork", bufs=2) as pw, \
                 tc.tile_pool(name="plnp", bufs=3) as plnp:
                cpmt_sb = pp.tile([128, MT, 2], f32)
                nc.sync.dma_start(out=cpmt_sb, in_=cpmt)
                b2fr_sb = pp.tile([1, DIM], f32r)
                nc.sync.dma_start(out=b2fr_sb, in_=b2fr)
                h1T = pp.tile([128, 4, M], bf16)
                with tc.tile_pool(name="pdep", bufs=1) as pdp, \
                     tc.tile_pool(name="psd", bufs=2, space="PSUM") as psdp:
                    depth_sb = pdp.tile([1, M], f32r)
                    nc.sync.dma_start(out=depth_sb, in_=depth)
                    for (o, w) in TOKT:
                        psd = psdp.tile([128, 512], f32, tag="psd")
                        nc.tensor.matmul(psd[:, :w], onesrow, depth_sb[:, o:o + w], start=True, stop=True)
                        for fc in range(4):
                            nc.scalar.activation(h1T[:, fc, o:o + w], psd[:, :w], AF.Gelu,
                                                 scale=w1pp_sb[:, fc:fc + 1], bias=b1pp_sb[:, fc:fc + 1])
                d2f_c = []
                for fc in range(4):
                    t = pp.tile([128, DIM], bf16, tag=f"d2f{fc}")
                    nc.sync.dma_start(out=t, in_=d2fw2[fc * 128:(fc + 1) * 128, :])
                    d2f_c.append(t)
                with tc.tile_pool(name="pemb", bufs=3, space="PSUM") as pembp, \
                     tc.tile_pool(name="ptr", bufs=3, space="PSUM") as ptrp:
                    for mt in range(MT):
                        # positional-encoding args for this token chunk
                        ps_arg = pe_arg(pw, cpmt_sb[:, mt, :], 128, "parg")
                        ctx_c = pw.tile([128, DIM], f32, tag="ctxc")
                        sincos(pw, ps_arg, 128, ctx_c)
                        # prompt_emb (d2f) for this chunk, add into ctx
                        for dh in range(2):
                            ps_e = pembp.tile([128, 512], f32, tag="pemb")
                            for fc in range(4):
                                nc.tensor.matmul(ps_e, h1T[:, fc, mt * 128:(mt + 1) * 128],
                                                 d2f_c[fc][:, dh * 512:(dh + 1) * 512],
                                                 start=(fc == 0), stop=False)
                            nc.tensor.matmul(ps_e, onesrow, b2fr_sb[:, dh * 512:(dh + 1) * 512],
                                             start=False, stop=True)
                            nc.vector.tensor_tensor(out=ctx_c[:, dh * 512:(dh + 1) * 512],
                                                    in0=ctx_c[:, dh * 512:(dh + 1) * 512], in1=ps_e, op=ALU.add)
                        cn_c = pw.tile([128, DIM], f32r, tag="cnc")
                        ln_apply(plnp, ctx_c, 128, cn_c)
                        for fc in range(8):
                            ps_t = ptrp.tile([128, 128], f32r, tag="ptr")
                            nc.tensor.transpose(ps_t, cn_c[:, fc * 128:(fc + 1) * 128], ident)
                            if fc % 2 == 0:
                                nc.vector.tensor_copy(cnT[:, fc, mt * 128:(mt + 1) * 128], ps_t)
                            else:
                                nc.scalar.copy(cnT[:, fc, mt * 128:(mt + 1) * 128], ps_t)

            # ---------------- phase K: K/V projection (spilled to DRAM) ----------
            with tc.tile_pool(name="kvv", bufs=1) as kvv, \
                 tc.tile_pool(name="kst", bufs=3) as kstp, \
                 tc.tile_pool(name="pk", bufs=3, space="PSUM") as pkp:
                bvB = loadvec(kvv, "bv_v")
                with tc.tile_pool(name="wkp", bufs=1) as wkp:
                    wk_c = []
                    for kc in range(8):
                        t = wkp.tile([128, DIM], f32r, tag=f"wk{kc}")
                        nc.sync.dma_start(out=t, in_=wk[kc * 128:(kc + 1) * 128, :])
                        wk_c.append(t)
                    for fc in range(8):
                        for (o, w) in TOKT:
                            ps_k = pkp.tile([128, 512], f32, tag="pk")
                            for kc in range(8):
                                nc.tensor.matmul(ps_k[:, :w], wk_c[kc][:, fc * 128:(fc + 1) * 128],
                                                 cnT[:, kc, o:o + w], start=(kc == 0), stop=(kc == 7))
                            kst = kstp.tile([128, 512], f32r, tag="kst")
                            nc.vector.tensor_scalar(kst[:, :w], ps_k[:, :w], bkpp_sb[:, fc:fc + 1], None, op0=ALU.add)
                            nc.sync.dma_start(out=kscr[fc, :, o:o + w], in_=kst[:, :w])
                with tc.tile_pool(name="wvp", bufs=1) as wvp:
                    wv_c = []
                    for kc in range(8):
                        t = wvp.tile([128, DIM], f32r, tag=f"wv{kc}")
                        nc.sync.dma_start(out=t, in_=wv[kc * 128:(kc + 1) * 128, :])
                        wv_c.append(t)
                    for mt in range(MT):
                        vst = kstp.tile([128, DIM], bf16, tag="vst")
                        for dh in range(2):
                            ps_v = pkp.tile([128, 512], f32, tag="pv")
                            for kc in range(8):
                                nc.tensor.matmul(ps_v, cnT[:, kc, mt * 128:(mt + 1) * 128],
                                                 wv_c[kc][:, dh * 512:(dh + 1) * 512],
                                                 start=(kc == 0), stop=(kc == 7))
                            nc.vector.tensor_tensor(out=vst[:, dh * 512:(dh + 1) * 512], in0=ps_v,
                                                    in1=bvB[:, dh * 512:(dh + 1) * 512], op=ALU.add)
                        nc.sync.dma_start(out=vscr[mt * 128:(mt + 1) * 128, :], in_=vst)
                    nc.sync.dma_start(out=dbg_cn, in_=cnT[:, :, 0:128])

        qo = tc.alloc_tile_pool(name="qopool", bufs=1)
        qT = qo.tile([128, 8, NQ], f32r)
        oT = qo.tile([128, 8, NQ], f32r)

        # ---------------- phase X: image path -> x_sb, qT --------------------
        with tc.tile_pool(name="xw", bufs=1) as xw, \
             tc.tile_pool(name="xwork", bufs=3) as xwk, \
             tc.tile_pool(name="xlnp", bufs=3) as xlnp:
            xnT = xw.tile([128, 8, NQ], f32r)
            with tc.tile_pool(name="xnrm", bufs=1) as xnrmp, \
                 tc.tile_pool(name="xtr", bufs=4, space="PSUM") as xtrp:
                xn_rm = xnrmp.tile([128, 6, DIM], f32r)
                for rc, (o, p) in enumerate(RC):
                    nc.sync.dma_start(out=x_sb[:p, rc, :], in_=ximg[o:o + p, :])
                    ps_arg = pe_arg(xwk, cimg_sb[:, rc, :], p, "xarg")
                    pe_c = xwk.tile([128, DIM], f32, tag="pec")
                    sincos(xwk, ps_arg[:p], p, pe_c)
                    nc.vector.tensor_tensor(out=x_sb[:p, rc, :], in0=x_sb[:p, rc, :], in1=pe_c[:p], op=ALU.add)
                    ln_apply(xlnp, x_sb[:p, rc, :], p, xn_rm[:, rc, :])
                for rc, (o, p) in enumerate(RC):
                    for fc in range(8):
                        ps_t = xtrp.tile([128, 128], f32r, tag="xtr")
                        nc.tensor.transpose(ps_t[:, :p], xn_rm[:p, rc, fc * 128:(fc + 1) * 128], ident[:p, :p])
                        if fc % 2 == 0:
                            nc.vector.tensor_copy(xnT[:, fc, o:o + p], ps_t[:, :p])
                        else:
                            nc.scalar.copy(xnT[:, fc, o:o + p], ps_t[:, :p])
                nc.sync.dma_start(out=dbg_x, in_=x_sb[:, 0, :])
                nc.sync.dma_start(out=dbg_xn, in_=xnT[:, :, 0:128])
            with tc.tile_pool(name="wqp", bufs=1) as wqp, \
                 tc.tile_pool(name="pq", bufs=3, space="PSUM") as pqp:
                wq_c = []
                for kc in range(8):
                    t = wqp.tile([128, DIM], f32r, tag=f"wq{kc}")
                    nc.sync.dma_start(out=t, in_=wq[kc * 128:(kc + 1) * 128, :])
                    wq_c.append(t)
                for fc in range(8):
                    for qt in range(2):
                        ps_q = pqp.tile([128, QT], f32, tag="pq")
                        for kc in range(8):
                            nc.tensor.matmul(ps_q, wq_c[kc][:, fc * 128:(fc + 1) * 128],
                                             xnT[:, kc, qt * QT:(qt + 1) * QT], start=(kc == 0), stop=(kc == 7))
                        nc.vector.tensor_scalar(qT[:, fc, qt * QT:(qt + 1) * QT], ps_q,
                                                bqpp_sb[:, fc:fc + 1], None, op0=ALU.add)
                nc.sync.dma_start(out=dbg_q, in_=qT[:, :, 0:128])
                nc.sync.dma_start(out=dbg_k, in_=kscr[0, :, 0:1024])
                nc.sync.dma_start(out=dbg_v, in_=vscr[0:128, :])

        # ---------------- phase A: attention ---------------------------------
        with tc.tile_pool(name="ah", bufs=2) as ahp, \
             tc.tile_pool(name="aex", bufs=2) as aexp, \
             tc.tile_pool(name="awk", bufs=3) as awk, \
             tc.tile_pool(name="pl", bufs=3, space="PSUM") as plp, \
             tc.tile_pool(name="psm", bufs=1, space="PSUM") as psmp, \
             tc.tile_pool(name="pav", bufs=3, space="PSUM") as pavp:
            for h in range(HEADS):
                kh = ahp.tile([128, 2, M], f32r, tag="kh")
                nc.sync.dma_start(out=kh, in_=kscr[2 * h:2 * h + 2].rearrange("c p m -> p c m"))
                vh = ahp.tile([128, MT, HEAD], bf16, tag="vh")
                nc.sync.dma_start(out=vh, in_=vscr.rearrange("(mt p) d -> p mt d", p=128)[:, :, h * HEAD:(h + 1) * HEAD])
                for qt in range(2):
                    expT = aexp.tile([128, MT, QT], bf16, tag="expT")
                    for mt in range(MT):
                        ps_l = plp.tile([128, QT], f32, tag="pl")
                        for c in range(2):
                            nc.tensor.matmul(ps_l, kh[:, c, mt * 128:(mt + 1) * 128],
                                             qT[:, 2 * h + c, qt * QT:(qt + 1) * QT],
                                             start=(c == 0), stop=(c == 1))
                        nc.scalar.activation(expT[:, mt, :], ps_l, AF.Exp,
                                             scale=0.0625, bias=mbias[:, mt:mt + 1])
                    ps_s = psmp.tile([1, QT], f32, tag="ps_s")
                    for mt in range(MT):
                        nc.tensor.matmul(ps_s, ones128b, expT[:, mt, :], start=(mt == 0), stop=(mt == MT - 1))
                    rr = awk.tile([1, QT], f32r, tag="rr")
                    with nc.allow_low_precision(reason="softmax denom reciprocal in f32r (~2^-12)"):
                        nc.vector.reciprocal(rr, ps_s)
                    ps_rb = psmp.tile([128, QT], f32, tag="ps_rb")
                    nc.tensor.matmul(ps_rb, onesrow, rr, start=True, stop=True)
                    recipB = awk.tile([128, QT], f32, tag="recipB")
                    nc.vector.tensor_copy(recipB, ps_rb)
                    for c in range(2):
                        ps_o = pavp.tile([128, QT], f32, tag="ps_o")
                        for mt in range(MT):
                            nc.tensor.matmul(ps_o, vh[:, mt, c * 128:(c + 1) * 128], expT[:, mt, :],
                                             start=(mt == 0), stop=(mt == MT - 1))
                        nc.vector.tensor_tensor(out=oT[:, 2 * h + c, qt * QT:(qt + 1) * QT],
                                                in0=ps_o, in1=recipB, op=ALU.mult)

            nc.sync.dma_start(out=dbg_o, in_=oT[:, :, 0:128])

        # attention out-projection + residual (row-major, into x_sb)
        with tc.tile_pool(name="wop", bufs=1) as wop, \
             tc.tile_pool(name="aprw", bufs=3) as aprw, \
             tc.tile_pool(name="pa", bufs=4, space="PSUM") as pap:
            boB = loadvec(wop, "bo_v")
            wo_c = []
            for fc in range(8):
                t = wop.tile([128, DIM], f32r, tag=f"wo{fc}")
                nc.sync.dma_start(out=t, in_=wo[fc * 128:(fc + 1) * 128, :])
                wo_c.append(t)
            for rc, (o, p) in enumerate(RC):
                for dh in range(2):
                    ps_a = pap.tile([128, 512], f32, tag="pa")
                    for fc in range(8):
                        nc.tensor.matmul(ps_a[:p], oT[:, fc, o:o + p],
                                         wo_c[fc][:, dh * 512:(dh + 1) * 512], start=(fc == 0), stop=(fc == 7))
                    t_a = aprw.tile([128, 512], f32, tag="ta")
                    nc.vector.tensor_tensor(out=t_a[:p], in0=ps_a[:p],
                                            in1=boB[:p, dh * 512:(dh + 1) * 512], op=ALU.add)
                    nc.vector.tensor_tensor(out=x_sb[:p, rc, dh * 512:(dh + 1) * 512],
                                            in0=x_sb[:p, rc, dh * 512:(dh + 1) * 512], in1=t_a[:p], op=ALU.add)

        qo.release()

        # ---------------- phase M: MLP on x2 ---------------------------------
        with tc.tile_pool(name="mw2", bufs=1) as mw2p, \
             tc.tile_pool(name="mmain", bufs=1) as mmain, \
             tc.tile_pool(name="mwork", bufs=2) as mwork:
            w2_c = []
            for hc in range(32):
                t = mw2p.tile([128, DIM], bf16, tag=f"w2{hc}")
                nc.sync.dma_start(out=t, in_=w2b[hc * 128:(hc + 1) * 128, :])
                w2_c.append(t)
            b2mB = loadvec(mmain, "b2m_v")
            x2nT = mmain.tile([128, 8, NQ], f32r)
            with tc.tile_pool(name="mlnv", bufs=1) as mlnv, \
                 tc.tile_pool(name="mlnp", bufs=3) as mlnp, \
                 tc.tile_pool(name="mtr", bufs=4, space="PSUM") as mtrp:
                x2n_rm = mlnv.tile([128, 6, DIM], f32r)
                for rc, (o, p) in enumerate(RC):
                    ln_apply(mlnp, x_sb[:p, rc, :], p, x2n_rm[:, rc, :])
                for rc, (o, p) in enumerate(RC):
                    for fc in range(8):
                        ps_t = mtrp.tile([128, 128], f32r, tag="mtr")
                        nc.tensor.transpose(ps_t[:, :p], x2n_rm[:p, rc, fc * 128:(fc + 1) * 128], ident[:p, :p])
                        if fc % 2 == 0:
                            nc.vector.tensor_copy(x2nT[:, fc, o:o + p], ps_t[:, :p])
                        else:
                            nc.scalar.copy(x2nT[:, fc, o:o + p], ps_t[:, :p])
            hTp = tc.alloc_tile_pool(name="hTp", bufs=1)
            hT = hTp.tile([128, 32, NQ], bf16)
            with tc.tile_pool(name="w1p", bufs=2) as w1p, \
                 tc.tile_pool(name="ph", bufs=4, space="PSUM") as php:
                for e in range(16):  # stream w1 in 1MB column blocks (2 h-chunks each)
                    w1_sb = w1p.tile([128, 8, 256], f32r, tag="w1s")
                    nc.sync.dma_start(out=w1_sb,
                                      in_=w1.rearrange("(c p) d -> p c d", p=128)[:, :, e * 256:(e + 1) * 256])
                    for hcl in range(2):
                        hc = e * 2 + hcl
                        for qt in range(2):
                            ps_h = php.tile([128, QT], f32, tag="ph")
                            for kc in range(8):
                                nc.tensor.matmul(ps_h, w1_sb[:, kc, hcl * 128:(hcl + 1) * 128],
                                                 x2nT[:, kc, qt * QT:(qt + 1) * QT],
                                                 start=(kc == 0), stop=(kc == 7))
                            nc.scalar.activation(hT[:, hc, qt * QT:(qt + 1) * QT], ps_h, AF.Gelu,
                                                 bias=b1mpp_sb[:, hc:hc + 1])
            with tc.tile_pool(name="po", bufs=4, space="PSUM") as pop:
                for rc, (o, p) in enumerate(RC):
                    for dh in range(2):
                        ps_f = pop.tile([128, 512], f32, tag="po")
                        for hc in range(32):
                            nc.tensor.matmul(ps_f[:p], hT[:, hc, o:o + p],
                                             w2_c[hc][:, dh * 512:(dh + 1) * 512],
                                             start=(hc == 0), stop=(hc == 31))
                        o_f = mwork.tile([128, 512], f32, tag="of")
                        nc.vector.tensor_tensor(out=o_f[:p], in0=ps_f[:p],
                                                in1=b2mB[:p, dh * 512:(dh + 1) * 512], op=ALU.add)
                        nc.vector.tensor_tensor(out=o_f[:p], in0=o_f[:p],
                                                in1=x_sb[:p, rc, dh * 512:(dh + 1) * 512], op=ALU.add)
                        nc.sync.dma_start(out=out[o:o + p, dh * 512:(dh + 1) * 512], in_=o_f[:p])

            hTp.release()
        xp.release()
        cp.release()

    nc.finalize()
    return nc


def _gridc(h, w):
    x = (np.arange(w, dtype=np.float64) + 0.5) / w
    y = (np.arange(h, dtype=np.float64) + 0.5) / h
    xx, yy = np.meshgrid(x, y)
    return np.stack([xx.ravel(), yy.ravel()], 0)  # [2, h*w], (x, y)


def _run(inputs, trace=False):
    global _cached
    if _cached is None:
        _cached = _build()
    nc = _cached

    p = {k: np.asarray(v, np.float32) for k, v in inputs["params"].items()}
    image = np.asarray(inputs["image_embeddings"], np.float32)
    pdepth = np.asarray(inputs["prompt_depth"], np.float32)
    pmask = np.asarray(inputs["prompt_mask"], np.float32)

    cimg_full = (2 * PI * (2 * _gridc(37, 37) - 1)).astype(np.float32)  # [2, 1369]
    cpmt_flat = (2 * PI * (2 * _gridc(48, 48) - 1)).astype(np.float32)  # [2, 2304]
    cpmt = np.ascontiguousarray(cpmt_flat.T.reshape(MT, 128, 2).transpose(1, 0, 2))  # [128, MT, 2]

    f64 = np.float64
    wq_f = p["ax_wq"].astype(f64) * p["n1x_g"].astype(f64)[:, None]
    bq_f = p["ax_bq"].astype(f64) + p["n1x_b"].astype(f64) @ p["ax_wq"].astype(f64)
    wk_f = p["ax_wkv"][:, :DIM].astype(f64) * p["n1c_g"].astype(f64)[:, None]
    bk_f = p["ax_bkv"][:DIM].astype(f64) + p["n1c_b"].astype(f64) @ p["ax_wkv"][:, :DIM].astype(f64)
    wv_f = p["ax_wkv"][:, DIM:].astype(f64) * p["n1c_g"].astype(f64)[:, None]
    bv_f = p["ax_bkv"][DIM:].astype(f64) + p["n1c_b"].astype(f64) @ p["ax_wkv"][:, DIM:].astype(f64)
    w1_f = p["mx_w1"].astype(f64) * p["n2x_g"].astype(f64)[:, None]
    b1m_f = p["mx_b1"].astype(f64) + p["n2x_b"].astype(f64) @ p["mx_w1"].astype(f64)
    shared = {
        "cpmt": cpmt,
        "gauss": np.ascontiguousarray(p["pe_gauss"]),
        "wq": np.ascontiguousarray(wq_f.astype(np.float32)),
        "wk": np.ascontiguousarray(wk_f.astype(np.float32)),
        "wv": np.ascontiguousarray(wv_f.astype(np.float32)),
        "wo": np.ascontiguousarray(p["ax_wo"]),
        "w1": np.ascontiguousarray(w1_f.astype(np.float32)),
        "w2b": p["mx_w2"].astype(ml_dtypes.bfloat16),
        "d2fw2": p["d2f_w2"].astype(ml_dtypes.bfloat16),
        "w1pp": np.ascontiguousarray(p["d2f_w1"].reshape(4, 128).T),
        "b1pp": np.ascontiguousarray(p["d2f_b1"].reshape(4, 128).T),
        "bqpp": np.ascontiguousarray(bq_f.astype(np.float32).reshape(8, 128).T),
        "bkpp": np.ascontiguousarray(bk_f.astype(np.float32).reshape(8, 128).T),
        "b1mpp": np.ascontiguousarray(b1m_f.astype(np.float32).reshape(32, 128).T),
        "n1cg": p["n1c_g"][None], "n1cb": p["n1c_b"][None],
        "n1xg": p["n1x_g"][None], "n1xb": p["n1x_b"][None],
        "n2xg": p["n2x_g"][None], "n2xb": p["n2x_b"][None],
        "bo_v": p["ax_bo"][None], "bv_v": bv_f.astype(np.float32)[None],
        "b2f_v": p["d2f_b2"][None], "b2m_v": p["mx_b2"][None],
        "b2fr": np.ascontiguousarray(p["d2f_b2"][None]),
    }
    in_maps = []
    for c in range(8):
        b, half = c // 2, c % 2
        rows = slice(0, NQ) if half == 0 else slice(NFULL - NQ, NFULL)
        m = dict(shared)
        m["ximg"] = np.ascontiguousarray(image[b, rows])
        cr = np.zeros((768, 2), np.float32)
        cr[:NQ] = cimg_full[:, rows].T
        m["cimg"] = np.ascontiguousarray(cr.reshape(6, 128, 2).transpose(1, 0, 2))  # [128, 6, 2]
        m["depth"] = np.ascontiguousarray(pdepth[b, 0].reshape(1, M))
        m["maskv"] = np.ascontiguousarray(pmask[b, 0].reshape(M))
        in_maps.append(m)

    res = run_bass_kernel_spmd(nc, in_maps, list(range(8)), trace=trace,
                               stitch_traces=trace)
    full = np.empty((B, NFULL, DIM), np.float32)
    for b in range(B):
        full[b, :NFULL - NQ] = res.results[2 * b]["out"][:NFULL - NQ]
        full[b, NFULL - NQ:] = res.results[2 * b + 1]["out"]
    return full, res


def kernel(**inputs):
    full, _ = _run(inputs, trace=False)
    return full


# revision 21
# speedup vs baseline: 1.5336x; 1.0486x over previous
"""Trainium2 Bass kernel for nn_CrossAttnPromptModel (8-core SPMD).

Sharding: core c -> (batch b=c//2, row-half c%2). Each core processes 688 of the
1369 image tokens for its batch (halves overlap by 7 rows); prompt-side work
(d2f MLP, ctx LN, K/V projection) is duplicated across the pair. The second
cross-attention and the ctx MLP do not affect the returned output (dead code)
and are skipped. Matmuls run in fp32r (TF32-like, full PE rate); the
attention AV and MLP second matmul run in bf16 to fit SBUF.
"""
import sys
if '/opt/trn_rl_repo' not in sys.path:
    sys.path.insert(0, '/opt/trn_rl_repo')
import numpy as np
import ml_dtypes
import concourse.bass as bass
import concourse.mybir as mybir
import concourse.tile as tile
from concourse import bacc
from concourse.bass_utils import run_bass_kernel_spmd
from concourse.masks import make_identity

f32 = mybir.dt.float32
f32r = mybir.dt.float32r
bf16 = mybir.dt.bfloat16
AF = mybir.ActivationFunctionType
ALU = mybir.AluOpType

PI = float(np.pi)
MAGIC = 1.5 * 2.0 ** 23
B, NFULL, DIM, HEADS, HEAD, PE = 4, 1369, 1024, 4, 256, 512
M, MT = 2304, 18
NQ, QT = 688, 344
RC = [(0, 128), (128, 128), (256, 128), (384, 128), (512, 128), (640, 48)]
TOKT = [(0, 512), (512, 512), (1024, 512), (1536, 512), (2048, 256)]
LN_EPS = 1e-5

_cached = None


def _build():
    nc = bacc.Bacc("TRN2", target_bir_lowering=False, debug=False)

    def din(name, shape, dt):
        return nc.dram_tensor(name, shape, dt, kind="ExternalInput").ap()

    ximg = din("ximg", [NQ, DIM], f32)
    cimg = din("cimg", [128, 6, 2], f32)
    cpmt = din("cpmt", [128, MT, 2], f32)
    gauss = din("gauss", [2, PE], f32)
    depth = din("depth", [1, M], f32r)
    maskv = din("maskv", [M], f32)
    wq = din("wq", [DIM, DIM], f32r)
    wk = din("wk", [DIM, DIM], f32r)
    wv = din("wv", [DIM, DIM], f32r)
    wo = din("wo", [DIM, DIM], f32r)
    w1 = din("w1", [DIM, 4 * DIM], f32r)
    w2b = din("w2b", [4 * DIM, DIM], bf16)
    b2fr = din("b2fr", [1, DIM], f32r)
    d2fw2 = din("d2fw2", [PE, DIM], bf16)
    w1pp = din("w1pp", [128, 4], f32)
    b1pp = din("b1pp", [128, 4], f32)
    bqpp = din("bqpp", [128, 8], f32)
    bkpp = din("bkpp", [128, 8], f32)
    b1mpp = din("b1mpp", [128, 32], f32)
    VNAMES = ["n1cg", "n1cb", "n1xg", "n1xb", "n2xg", "n2xb", "bo_v", "bv_v", "b2f_v", "b2m_v"]
    vecs = {n: din(n, [1, DIM], f32) for n in VNAMES}
    out = nc.dram_tensor("out", [NQ, DIM], f32, kind="ExternalOutput").ap()
    dbg_cn = nc.dram_tensor("dbg_cn", [128, 8, 128], f32r, kind="ExternalOutput").ap()
    dbg_x = nc.dram_tensor("dbg_x", [128, DIM], f32, kind="ExternalOutput").ap()
    dbg_xn = nc.dram_tensor("dbg_xn", [128, 8, 128], f32r, kind="ExternalOutput").ap()
    dbg_q = nc.dram_tensor("dbg_q", [128, 8, 128], f32r, kind="ExternalOutput").ap()
    dbg_o = nc.dram_tensor("dbg_o", [128, 8, 128], f32r, kind="ExternalOutput").ap()
    dbg_k = nc.dram_tensor("dbg_k", [128, 1024], f32r, kind="ExternalOutput").ap()
    dbg_v = nc.dram_tensor("dbg_v", [128, DIM], bf16, kind="ExternalOutput").ap()
    kscr = nc.dram_tensor("kscr", [8, 128, M], f32r).ap()
    vscr = nc.dram_tensor("vscr", [M, DIM], bf16).ap()

    with tile.TileContext(nc) as tc:
        cp = tc.alloc_tile_pool(name="consts", bufs=1)
        ident_f = cp.tile([128, 128], f32)
        make_identity(nc, ident_f)
        ident = cp.tile([128, 128], f32r)
        nc.vector.tensor_copy(ident, ident_f)
        ones_f = cp.tile([128, 1], f32)
        nc.vector.memset(ones_f, 1.0)
        ones128b = cp.tile([128, 1], bf16)
        nc.vector.tensor_copy(ones128b, ones_f)
        onesrow_f = cp.tile([1, 128], f32)
        nc.vector.memset(onesrow_f, 1.0)
        onesrow = cp.tile([1, 128], f32r)
        nc.vector.tensor_copy(onesrow, onesrow_f)
        epst = cp.tile([128, 1], f32)
        nc.vector.memset(epst, LN_EPS)
        halfpi = cp.tile([128, 1], f32)
        nc.vector.memset(halfpi, PI / 2)
        g0B = cp.tile([128, PE], f32)
        nc.sync.dma_start(out=g0B, in_=gauss[0:1, :].broadcast_to([128, PE]))
        g1B = cp.tile([128, PE], f32)
        nc.sync.dma_start(out=g1B, in_=gauss[1:2, :].broadcast_to([128, PE]))
        cimg_sb = cp.tile([128, 6, 2], f32)
        nc.sync.dma_start(out=cimg_sb, in_=cimg)
        w1pp_sb = cp.tile([128, 4], f32)
        nc.sync.dma_start(out=w1pp_sb, in_=w1pp)
        b1pp_sb = cp.tile([128, 4], f32)
        nc.sync.dma_start(out=b1pp_sb, in_=b1pp)
        bqpp_sb = cp.tile([128, 8], f32)
        nc.sync.dma_start(out=bqpp_sb, in_=bqpp)
        bkpp_sb = cp.tile([128, 8], f32)
        nc.sync.dma_start(out=bkpp_sb, in_=bkpp)
        b1mpp_sb = cp.tile([128, 32], f32)
        nc.sync.dma_start(out=b1mpp_sb, in_=b1mpp)
        maskT = cp.tile([128, MT], f32)
        nc.sync.dma_start(out=maskT, in_=maskv.rearrange("(mt p) -> p mt", p=128))
        mbias = cp.tile([128, MT], f32)
        nc.vector.tensor_scalar(mbias, maskT, 1e-6, None, op0=ALU.min)
        nc.vector.tensor_scalar(mbias, mbias, 1e-6, 1e15, op0=ALU.subtract, op1=ALU.mult)

        def loadvec(pool, name):
            t = pool.tile([128, DIM], f32, tag=name)
            nc.sync.dma_start(out=t, in_=vecs[name].broadcast_to([128, DIM]))
            return t

        def pe_arg(pool, cpp, p, tag):
            arg = pool.tile([128, PE], f32, tag=tag + "a")
            t2 = pool.tile([128, PE], f32, tag=tag + "b")
            nc.vector.tensor_scalar(arg[:p], g0B[:p], cpp[:p, 0:1], None, op0=ALU.mult)
            nc.vector.tensor_scalar(t2[:p], g1B[:p], cpp[:p, 1:2], None, op0=ALU.mult)
            nc.vector.tensor_tensor(out=arg[:p], in0=arg[:p], in1=t2[:p], op=ALU.add)
            return arg

        def sincos(pool, ps_arg, p, dst):
            # dst[:p, 0:512] = sin(arg); dst[:p, 512:1024] = cos(arg)
            t1 = pool.tile([128, PE], f32, tag="pet1")
            t2 = pool.tile([128, PE], f32, tag="pet2")
            z = pool.tile([128, PE], f32, tag="pez")
            nc.vector.tensor_scalar(t1[:p], ps_arg, 1.0 / (2 * PI), MAGIC, op0=ALU.mult, op1=ALU.add)
            nc.vector.tensor_scalar(t2[:p], t1[:p], MAGIC, 2 * PI, op0=ALU.subtract, op1=ALU.mult)
            nc.vector.tensor_tensor(out=z[:p], in0=ps_arg, in1=t2[:p], op=ALU.subtract)
            nc.scalar.activation(dst[:p, 0:PE], z[:p], AF.Sin)
            # cos(x) = cos(z) = sin(pi/2 - |z|); pi/2-|z| stays in the accurate Sin range
            az = pool.tile([128, PE], f32, tag="paz")
            nc.scalar.activation(az[:p], z[:p], AF.Abs)
            nc.scalar.activation(dst[:p, PE:2 * PE], az[:p], AF.Sin, scale=-1.0, bias=halfpi[:p])

        def ln_apply(pool, src, p, dst):
            # row-major layernorm (affine folded into downstream weights):
            # dst = (src - mean) * rstd, f32 -> f32r
            stats = pool.tile([128, 2, nc.vector.BN_STATS_DIM], f32, tag="lnstats")
            nc.vector.bn_stats(out=stats[:p, 0, :], in_=src[:, 0:512])
            nc.vector.bn_stats(out=stats[:p, 1, :], in_=src[:, 512:1024])
            mv = pool.tile([128, nc.vector.BN_AGGR_DIM], f32, tag="lnmv")
            nc.vector.bn_aggr(out=mv[:p], in_=stats[:p])
            sd = pool.tile([128, 1], f32, tag="lnsd")
            nc.scalar.activation(sd[:p], mv[:p, 1:2], AF.Sqrt, bias=epst[:p])
            rstd = pool.tile([128, 1], f32, tag="lnrstd")
            nc.vector.reciprocal(rstd[:p], sd[:p])
            nc.vector.tensor_scalar(dst[:p], src, mv[:p, 0:1], rstd[:p], op0=ALU.subtract, op1=ALU.mult)

        xp = tc.alloc_tile_pool(name="xpool", bufs=1)
        x_sb = xp.tile([128, 6, DIM], f32)

        # ---------------- phase P: prompt path -> cnT ----------------
        with tc.tile_pool(name="cnpool", bufs=1) as cnp:
            cnT = cnp.tile([128, 8, M], f32r)
            with tc.tile_pool(name="ppool", bufs=1) as pp, \
                 tc.tile_pool(name="pwork", bufs=2) as pw, \
                 tc.tile_pool(name="plnp", bufs=3) as plnp:
                cpmt_sb = pp.tile([128, MT, 2], f32)
                nc.sync.dma_start(out=cpmt_sb, in_=cpmt)
                b2fr_sb = pp.tile([1, DIM], f32r)
                nc.sync.dma_start(out=b2fr_sb, in_=b2fr)
                h1T = pp.tile([128, 4, M], bf16)
                with tc.tile_pool(name="pdep", bufs=1) as pdp, \
                     tc.tile_pool(name="psd", bufs=2, space="PSUM") as psdp:
                    depth_sb = pdp.tile([1, M], f32r)
                    nc.sync.dma_start(out=depth_sb, in_=depth)
                    for (o, w) in TOKT:
                        psd = psdp.tile([128, 512], f32, tag="psd")
                        nc.tensor.matmul(psd[:, :w], onesrow, depth_sb[:, o:o + w], start=True, stop=True)
                        for fc in range(4):
                            nc.scalar.activation(h1T[:, fc, o:o + w], psd[:, :w], AF.Gelu,
                                                 scale=w1pp_sb[:, fc:fc + 1], bias=b1pp_sb[:, fc:fc + 1])
                d2f_c = []
                for fc in range(4):
                    t = pp.tile([128, DIM], bf16, tag=f"d2f{fc}")
                    nc.sync.dma_start(out=t, in_=d2fw2[fc * 128:(fc + 1) * 128, :])
                    d2f_c.append(t)
                with tc.tile_pool(name="pemb", bufs=3, space="PSUM") as pembp, \
                     tc.tile_pool(name="ptr", bufs=3, space="PSUM") as ptrp:
                    for mt in range(MT):
                        # positional-encoding args for this token chunk
                        ps_arg = pe_arg(pw, cpmt_sb[:, mt, :], 128, "parg")
                        ctx_c = pw.tile([128, DIM], f32, tag="ctxc")
                        sincos(pw, ps_arg, 128, ctx_c)
                        # prompt_emb (d2f) for this chunk, add into ctx
                        for dh in range(2):
                            ps_e = pembp.tile([128, 512], f32, tag="pemb")
                            for fc in range(4):
                                nc.tensor.matmul(ps_e, h1T[:, fc, mt * 128:(mt + 1) * 128],
                                                 d2f_c[fc][:, dh * 512:(dh + 1) * 512],
                                                 start=(fc == 0), stop=False)
                            nc.tensor.matmul(ps_e, onesrow, b2fr_sb[:, dh * 512:(dh + 1) * 512],
                                             start=False, stop=True)
                            nc.vector.tensor_tensor(out=ctx_c[:, dh * 512:(dh + 1) * 512],
                                                    in0=ctx_c[:, dh * 512:(dh + 1) * 512], in1=ps_e, op=ALU.add)
                        cn_c = pw.tile([128, DIM], f32r, tag="cnc")
                        ln_apply(plnp, ctx_c, 128, cn_c)
                        for fc in range(8):
                            ps_t = ptrp.tile([128, 128], f32r, tag="ptr")
                            nc.tensor.transpose(ps_t, cn_c[:, fc * 128:(fc + 1) * 128], ident)
                            if fc % 2 == 0:
                                nc.vector.tensor_copy(cnT[:, fc, mt * 128:(mt + 1) * 128], ps_t)
                            else:
                                nc.scalar.copy(cnT[:, fc, mt * 128:(mt + 1) * 128], ps_t)

            # ---------------- phase K: K/V projection (spilled to DRAM) ----------
            with tc.tile_pool(name="kvv", bufs=1) as kvv, \
                 tc.tile_pool(name="kst", bufs=3) as kstp, \
                 tc.tile_pool(name="pk", bufs=3, space="PSUM") as pkp:
                bvB = loadvec(kvv, "bv_v")
                with tc.tile_pool(name="wkp", bufs=1) as wkp:
                    wk_c = []
                    for kc in range(8):
                        t = wkp.tile([128, DIM], f32r, tag=f"wk{kc}")
                        nc.sync.dma_start(out=t, in_=wk[kc * 128:(kc + 1) * 128, :])
                        wk_c.append(t)
                    for fc in range(8):
                        for (o, w) in TOKT:
                            ps_k = pkp.tile([128, 512], f32, tag="pk")
                            for kc in range(8):
                                nc.tensor.matmul(ps_k[:, :w], wk_c[kc][:, fc * 128:(fc + 1) * 128],
                                                 cnT[:, kc, o:o + w], start=(kc == 0), stop=(kc == 7))
                            kst = kstp.tile([128, 512], f32r, tag="kst")
                            nc.vector.tensor_scalar(kst[:, :w], ps_k[:, :w], bkpp_sb[:, fc:fc + 1], None, op0=ALU.add)
                            nc.sync.dma_start(out=kscr[fc, :, o:o + w], in_=kst[:, :w])
                with tc.tile_pool(name="wvp", bufs=1) as wvp:
                    wv_c = []
                    for kc in range(8):
                        t = wvp.tile([128, DIM], f32r, tag=f"wv{kc}")
                        nc.sync.dma_start(out=t, in_=wv[kc * 128:(kc + 1) * 128, :])
                        wv_c.append(t)
                    for mt in range(MT):
                        vst = kstp.tile([128, DIM], bf16, tag="vst")
                        for dh in range(2):
                            ps_v = pkp.tile([128, 512], f32, tag="pv")
                            for kc in range(8):
                                nc.tensor.matmul(ps_v, cnT[:, kc, mt * 128:(mt + 1) * 128],
                                                 wv_c[kc][:, dh * 512:(dh + 1) * 512],
                                                 start=(kc == 0), stop=(kc == 7))
                            nc.vector.tensor_tensor(out=vst[:, dh * 512:(dh + 1) * 512], in0=ps_v,
                                                    in1=bvB[:, dh * 512:(dh + 1) * 512], op=ALU.add)
                        nc.sync.dma_start(out=vscr[mt * 128:(mt + 1) * 128, :], in_=vst)
                    nc.sync.dma_start(out=dbg_cn, in_=cnT[:, :, 0:128])

        qo = tc.alloc_tile_pool(name="qopool", bufs=1)
        qT = qo.tile([128, 8, NQ], f32r)
        oT = qo.tile([128, 8, NQ], f32r)

        # ---------------- phase X: image path -> x_sb, qT --------------------
        with tc.tile_pool(name="xw", bufs=1) as xw, \
             tc.tile_pool(name="xwork", bufs=3) as xwk, \
             tc.tile_pool(name="xlnp", bufs=3) as xlnp:
            xnT = xw.tile([128, 8, NQ], f32r)
            with tc.tile_pool(name="xnrm", bufs=1) as xnrmp, \
                 tc.tile_pool(name="xtr", bufs=4, space="PSUM") as xtrp:
                xn_rm = xnrmp.tile([128, 6, DIM], f32r)
                for rc, (o, p) in enumerate(RC):
                    nc.sync.dma_start(out=x_sb[:p, rc, :], in_=ximg[o:o + p, :])
                    ps_arg = pe_arg(xwk, cimg_sb[:, rc, :], p, "xarg")
                    pe_c = xwk.tile([128, DIM], f32, tag="pec")
                    sincos(xwk, ps_arg[:p], p, pe_c)
                    nc.vector.tensor_tensor(out=x_sb[:p, rc, :], in0=x_sb[:p, rc, :], in1=pe_c[:p], op=ALU.add)
                    ln_apply(xlnp, x_sb[:p, rc, :], p, xn_rm[:, rc, :])
                for rc, (o, p) in enumerate(RC):
                    for fc in range(8):
                        ps_t = xtrp.tile([128, 128], f32r, tag="xtr")
                        nc.tensor.transpose(ps_t[:, :p], xn_rm[:p, rc, fc * 128:(fc + 1) * 128], ident[:p, :p])
                        if fc % 2 == 0:
                            nc.vector.tensor_copy(xnT[:, fc, o:o + p], ps_t[:, :p])
                        else:
                            nc.scalar.copy(xnT[:, fc, o:o + p], ps_t[:, :p])
                nc.sync.dma_start(out=dbg_x, in_=x_sb[:, 0, :])
                nc.sync.dma_start(out=dbg_xn, in_=xnT[:, :, 0:128])
            with tc.tile_pool(name="wqp", bufs=1) as wqp, \
                 tc.tile_pool(name="pq", bufs=3, space="PSUM") as pqp:
                wq_c = []
                for kc in range(8):
                    t = wqp.tile([128, DIM], f32r, tag=f"wq{kc}")
                    nc.sync.dma_start(out=t, in_=wq[kc * 128:(kc + 1) * 128, :])
                    wq_c.append(t)
                for fc in range(8):
                    for qt in range(2):
                        ps_q = pqp.tile([128, QT], f32, tag="pq")
                        for kc in range(8):
                            nc.tensor.matmul(ps_q, wq_c[kc][:, fc * 128:(fc + 1) * 128],
                                             xnT[:, kc, qt * QT:(qt + 1) * QT], start=(kc == 0), stop=(kc == 7))
                        nc.vector.tensor_scalar(qT[:, fc, qt * QT:(qt + 1) * QT], ps_q,
                                                bqpp_sb[:, fc:fc + 1], None, op0=ALU.add)
                nc.sync.dma_start(out=dbg_q, in_=qT[:, :, 0:128])
                nc.sync.dma_start(out=dbg_k, in_=kscr[0, :, 0:1024])
                nc.sync.dma_start(out=dbg_v, in_=vscr[0:128, :])

        # ---------------- phase A: attention ---------------------------------
        with tc.tile_pool(name="ah", bufs=2) as ahp, \
             tc.tile_pool(name="aex", bufs=2) as aexp, \
             tc.tile_pool(name="awk", bufs=3) as awk, \
             tc.tile_pool(name="pl", bufs=3, space="PSUM") as plp, \
             tc.tile_pool(name="psm", bufs=1, space="PSUM") as psmp, \
             tc.tile_pool(name="pav", bufs=3, space="PSUM") as pavp:
            for h in range(HEADS):
                kh = ahp.tile([128, 2, M], f32r, tag="kh")
                nc.sync.dma_start(out=kh, in_=kscr[2 * h:2 * h + 2].rearrange("c p m -> p c m"))
                vh = ahp.tile([128, MT, HEAD], bf16, tag="vh")
                nc.sync.dma_start(out=vh, in_=vscr.rearrange("(mt p) d -> p mt d", p=128)[:, :, h * HEAD:(h + 1) * HEAD])
                for qt in range(2):
                    expT = aexp.tile([128, MT, QT], bf16, tag="expT")
                    for mt in range(MT):
                        ps_l = plp.tile([128, QT], f32, tag="pl")
                        for c in range(2):
                            nc.tensor.matmul(ps_l, kh[:, c, mt * 128:(mt + 1) * 128],
                                             qT[:, 2 * h + c, qt * QT:(qt + 1) * QT],
                                             start=(c == 0), stop=(c == 1))
                        nc.scalar.activation(expT[:, mt, :], ps_l, AF.Exp,
                                             scale=0.0625, bias=mbias[:, mt:mt + 1])
                    ps_s = psmp.tile([1, QT], f32, tag="ps_s")
                    for mt in range(MT):
                        nc.tensor.matmul(ps_s, ones128b, expT[:, mt, :], start=(mt == 0), stop=(mt == MT - 1))
                    rr = awk.tile([1, QT], f32r, tag="rr")
                    with nc.allow_low_precision(reason="softmax denom reciprocal in f32r (~2^-12)"):
                        nc.vector.reciprocal(rr, ps_s)
                    ps_rb = psmp.tile([128, QT], f32, tag="ps_rb")
                    nc.tensor.matmul(ps_rb, onesrow, rr, start=True, stop=True)
                    recipB = awk.tile([128, QT], f32, tag="recipB")
                    nc.vector.tensor_copy(recipB, ps_rb)
                    for c in range(2):
                        ps_o = pavp.tile([128, QT], f32, tag="ps_o")
                        for mt in range(MT):
                            nc.tensor.matmul(ps_o, vh[:, mt, c * 128:(c + 1) * 128], expT[:, mt, :],
                                             start=(mt == 0), stop=(mt == MT - 1))
                        nc.vector.tensor_tensor(out=oT[:, 2 * h + c, qt * QT:(qt + 1) * QT],
                                                in0=ps_o, in1=recipB, op=ALU.mult)

            nc.sync.dma_start(out=dbg_o, in_=oT[:, :, 0:128])

        # attention out-projection + residual (row-major, into x_sb)
        with tc.tile_pool(name="wop", bufs=1) as wop, \
             tc.tile_pool(name="aprw", bufs=3) as aprw, \
             tc.tile_pool(name="pa", bufs=4, space="PSUM") as pap:
            boB = loadvec(wop, "bo_v")
            wo_c = []
            for fc in range(8):
                t = wop.tile([128, DIM], f32r, tag=f"wo{fc}")
                nc.sync.dma_start(out=t, in_=wo[fc * 128:(fc + 1) * 128, :])
                wo_c.append(t)
            for rc, (o, p) in enumerate(RC):
                for dh in range(2):
                    ps_a = pap.tile([128, 512], f32, tag="pa")
                    for fc in range(8):
                        nc.tensor.matmul(ps_a[:p], oT[:, fc, o:o + p],
                                         wo_c[fc][:, dh * 512:(dh + 1) * 512], start=(fc == 0), stop=(fc == 7))
                    t_a = aprw.tile([128, 512], f32, tag="ta")
                    nc.vector.tensor_tensor(out=t_a[:p], in0=ps_a[:p],
                                            in1=boB[:p, dh * 512:(dh + 1) * 512], op=ALU.add)
                    nc.vector.tensor_tensor(out=x_sb[:p, rc, dh * 512:(dh + 1) * 512],
                                            in0=x_sb[:p, rc, dh * 512:(dh + 1) * 512], in1=t_a[:p], op=ALU.add)

        qo.release()

        # ---------------- phase M: MLP on x2 ---------------------------------
        with tc.tile_pool(name="mw2", bufs=1) as mw2p, \
             tc.tile_pool(name="mmain", bufs=1) as mmain, \
             tc.tile_pool(name="mwork", bufs=2) as mwork:
            w2_c = []
            for hc in range(32):
                t = mw2p.tile([128, DIM], bf16, tag=f"w2{hc}")
                nc.sync.dma_start(out=t, in_=w2b[hc * 128:(hc + 1) * 128, :])
                w2_c.append(t)
            b2mB = loadvec(mmain, "b2m_v")
            x2nT = mmain.tile([128, 8, NQ], f32r)
            with tc.tile_pool(name="mlnv", bufs=1) as mlnv, \
                 tc.tile_pool(name="mlnp", bufs=3) as mlnp, \
                 tc.tile_pool(name="mtr", bufs=4, space="PSUM") as mtrp:
                x2n_rm = mlnv.tile([128, 6, DIM], f32r)
                for rc, (o, p) in enumerate(RC):
                    ln_apply(mlnp, x_sb[:p, rc, :], p, x2n_rm[:, rc, :])
                for rc, (o, p) in enumerate(RC):
                    for fc in range(8):
                        ps_t = mtrp.tile([128, 128], f32r, tag="mtr")
                        nc.tensor.transpose(ps_t[:, :p], x2n_rm[:p, rc, fc * 128:(fc + 1) * 128], ident[:p, :p])
                        if fc % 2 == 0:
                            nc.vector.tensor_copy(x2nT[:, fc, o:o + p], ps_t[:, :p])
                        else:
                            nc.scalar.copy(x2nT[:, fc, o:o + p], ps_t[:, :p])
            hTp = tc.alloc_tile_pool(name="hTp", bufs=1)
            hT = hTp.tile([128, 32, NQ], bf16)
            with tc.tile_pool(name="w1p", bufs=2) as w1p, \
                 tc.tile_pool(name="ph", bufs=4, space="PSUM") as php:
                for e in range(16):  # stream w1 in 1MB column blocks (2 h-chunks each)
                    w1_sb = w1p.tile([128, 8, 256], f32r, tag="w1s")
                    nc.sync.dma_start(out=w1_sb,
                                      in_=w1.rearrange("(c p) d -> p c d", p=128)[:, :, e * 256:(e + 1) * 256])
                    for hcl in range(2):
                        hc = e * 2 + hcl
                        for qt in range(2):
                            ps_h = php.tile([128, QT], f32, tag="ph")
                            for kc in range(8):
                                nc.tensor.matmul(ps_h, w1_sb[:, kc, hcl * 128:(hcl + 1) * 128],
                                                 x2nT[:, kc, qt * QT:(qt + 1) * QT],
                                                 start=(kc == 0), stop=(kc == 7))
                            nc.scalar.activation(hT[:, hc, qt * QT:(qt + 1) * QT], ps_h, AF.Gelu,
                                                 bias=b1mpp_sb[:, hc:hc + 1])
            with tc.tile_pool(name="po", bufs=4, space="PSUM") as pop:
                for rc, (o, p) in enumerate(RC):
                    for dh in range(2):
                        ps_f = pop.tile([128, 512], f32, tag="po")
                        for hc in range(32):
                            nc.tensor.matmul(ps_f[:p], hT[:, hc, o:o + p],
                                             w2_c[hc][:, dh * 512:(dh + 1) * 512],
                                             start=(hc == 0), stop=(hc == 31))
                        o_f = mwork.tile([128, 512], f32, tag="of")
                        nc.vector.tensor_tensor(out=o_f[:p], in0=ps_f[:p],
                                                in1=b2mB[:p, dh * 512:(dh + 1) * 512], op=ALU.add)
                        nc.vector.tensor_tensor(out=o_f[:p], in0=o_f[:p],
                                                in1=x_sb[:p, rc, dh * 512:(dh + 1) * 512], op=ALU.add)
                        nc.sync.dma_start(out=out[o:o + p, dh * 512:(dh + 1) * 512], in_=o_f[:p])

            hTp.release()
        xp.release()
        cp.release()

    nc.finalize()
    return nc


def _gridc(h, w):
    x = (np.arange(w, dtype=np.float64) + 0.5) / w
    y = (np.arange(h, dtype=np.float64) + 0.5) / h
    xx, yy = np.meshgrid(x, y)
    return np.stack([xx.ravel(), yy.ravel()], 0)  # [2, h*w], (x, y)


def _run(inputs, trace=False):
    global _cached
    if _cached is None:
        _cached = _build()
    nc = _cached

    p = {k: np.asarray(v, np.float32) for k, v in inputs["params"].items()}
    image = np.asarray(inputs["image_embeddings"], np.float32)
    pdepth = np.asarray(inputs["prompt_depth"], np.float32)
    pmask = np.asarray(inputs["prompt_mask"], np.float32)

    cimg_full = (2 * PI * (2 * _gridc(37, 37) - 1)).astype(np.float32)  # [2, 1369]
    cpmt_flat = (2 * PI * (2 * _gridc(48, 48) - 1)).astype(np.float32)  # [2, 2304]
    cpmt = np.ascontiguousarray(cpmt_flat.T.reshape(MT, 128, 2).transpose(1, 0, 2))  # [128, MT, 2]

    f64 = np.float64
    wq_f = p["ax_wq"].astype(f64) * p["n1x_g"].astype(f64)[:, None]
    bq_f = p["ax_bq"].astype(f64) + p["n1x_b"].astype(f64) @ p["ax_wq"].astype(f64)
    wk_f = p["ax_wkv"][:, :DIM].astype(f64) * p["n1c_g"].astype(f64)[:, None]
    bk_f = p["ax_bkv"][:DIM].astype(f64) + p["n1c_b"].astype(f64) @ p["ax_wkv"][:, :DIM].astype(f64)
    wv_f = p["ax_wkv"][:, DIM:].astype(f64) * p["n1c_g"].astype(f64)[:, None]
    bv_f = p["ax_bkv"][DIM:].astype(f64) + p["n1c_b"].astype(f64) @ p["ax_wkv"][:, DIM:].astype(f64)
    w1_f = p["mx_w1"].astype(f64) * p["n2x_g"].astype(f64)[:, None]
    b1m_f = p["mx_b1"].astype(f64) + p["n2x_b"].astype(f64) @ p["mx_w1"].astype(f64)
    shared = {
        "cpmt": cpmt,
        "gauss": np.ascontiguousarray(p["pe_gauss"]),
        "wq": np.ascontiguousarray(wq_f.astype(np.float32)),
        "wk": np.ascontiguousarray(wk_f.astype(np.float32)),
        "wv": np.ascontiguousarray(wv_f.astype(np.float32)),
        "wo": np.ascontiguousarray(p["ax_wo"]),
        "w1": np.ascontiguousarray(w1_f.astype(np.float32)),
        "w2b": p["mx_w2"].astype(ml_dtypes.bfloat16),
        "d2fw2": p["d2f_w2"].astype(ml_dtypes.bfloat16),
        "w1pp": np.ascontiguousarray(p["d2f_w1"].reshape(4, 128).T),
        "b1pp": np.ascontiguousarray(p["d2f_b1"].reshape(4, 128).T),
        "bqpp": np.ascontiguousarray(bq_f.astype(np.float32).reshape(8, 128).T),
        "bkpp": np.ascontiguousarray(bk_f.astype(np.float32).reshape(8, 128).T),
        "b1mpp": np.ascontiguousarray(b1m_f.astype(np.float32).reshape(32, 128).T),
        "n1cg": p["n1c_g"][None], "n1cb": p["n1c_b"][None],
        "n1xg": p["n1x_g"][None], "n1xb": p["n1x_b"][None],
        "n2xg": p["n2x_g"][None], "n2xb": p["n2x_b"][None],
        "bo_v": p["ax_bo"][None], "bv_v": bv_f.astype(np.float32)[None],
        "b2f_v": p["d2f_b2"][None], "b2m_v": p["mx_b2"][None],
        "b2fr": np.ascontiguousarray(p["d2f_b2"][None]),
    }
    in_maps = []
    for c in range(8):
        b, half = c // 2, c % 2
        rows = slice(0, NQ) if half == 0 else slice(NFULL - NQ, NFULL)
        m = dict(shared)
        m["ximg"] = np.ascontiguousarray(image[b, rows])
        cr = np.zeros((768, 2), np.float32)
        cr[:NQ] = cimg_full[:, rows].T
        m["cimg"] = np.ascontiguousarray(cr.reshape(6, 128, 2).transpose(1, 0, 2))  # [128, 6, 2]
        m["depth"] = np.ascontiguousarray(pdepth[b, 0].reshape(1, M))
        m["maskv"] = np.ascontiguousarray(pmask[b, 0].reshape(M))
        in_maps.append(m)

    res = run_bass_kernel_spmd(nc, in_maps, list(range(8)), trace=trace,
                               stitch_traces=trace)
    full = np.empty((B, NFULL, DIM), np.float32)
    for b in range(B):
        full[b, :NFULL - NQ] = res.results[2 * b]["out"][:NFULL - NQ]
        full[b, NFULL - NQ:] = res.results[2 * b + 1]["out"]
    return full, res


def kernel(**inputs):
    full, _ = _run(inputs, trace=False)
    return full
